# revision 8
# baseline (speedup 1.0000x reference)
"""Trainium2 Bass kernel for the 2-layer LSTM (H=51 -> H=1) over T=2048 steps.

Data-parallel over batch: 8 cores x 128 batch, batch on the free dim,
state transposed (units on partitions). Per core/step all gate
pre-activations for BOTH layers land in one PSUM tile P (52, 512) as four
column blocks [I|G|F|O]; block col 51 is the (one-step-lagged) layer-2 cell.
tanh(z) = 2*sigmoid(2z) - 1 with the x2 folded into G weights, so sigmoid
covers all gates; the sigmoid is split [I,G]/[F,O] so the i*tanh(g) chain
starts before the F/O half finishes.

x_t enters through the state tile itself: row 53 of R holds x_t, loaded by
a GpSimd row-copy from an SBUF stripe that lives on partition 53 (DMA'd
from HBM 64 steps at a time), so each gate needs ONE matmul (K=54) instead
of a rank-1 update + matmul. R rotates over 4 tiles for DMA slack; y rows
leave via direct SBUF->HBM DMA each step.
"""

import numpy as np

H = 51
B = 128
NCORES = 8
N_FULL = 1024
T_FULL = 2048
XB = 64          # time steps per X stripe

_GATES = [(slice(102, 153), 2, 1.0),   # G (tanh applied directly)
          (slice(0, 51), 0, 1.0),      # I
          (slice(51, 102), 1, 1.0),    # F
          (slice(153, 204), 3, 1.0)]   # O


KR = 65      # K rows of R/A_ALL; x sits at partition 64 (engine ops need
XROW = 64    # quadrant-aligned partition starts, so 53 is not addressable)


def pack_weights(W_ih1, W_hh1, b_ih1, b_hh1, W_ih2, W_hh2, b_ih2, b_hh2):
    """lhsT pack. K rows: 0:51 h1, 51 h2, 52 const-1(bias), 64 x_t (53:64
    zero pad). M cols: four 52-wide gate blocks [I|G|F|O]; block col 51 =
    layer-2 cell."""
    blocks = []
    for l1_rows, l2_row, scale in _GATES:
        L = np.zeros((KR, 52), np.float32)
        L[0:51, 0:51] = W_hh1[l1_rows, :].T
        L[0:51, 51] = W_ih2[l2_row, :]
        L[51, 51] = W_hh2[l2_row, 0]
        L[52, 0:51] = b_ih1[l1_rows] + b_hh1[l1_rows]
        L[52, 51] = b_ih2[l2_row] + b_hh2[l2_row]
        L[XROW, 0:51] = W_ih1[l1_rows, 0]
        blocks.append(L * scale)
    return {"A_ALL": np.concatenate(blocks, axis=1)}  # (KR, 208)


def build_program(T=T_FULL, debug=False):
    import concourse.bass as bass
    import concourse.tile as tile
    from concourse import bacc, mybir

    dt = mybir.dt.float32
    nc = bacc.Bacc("TRN2", target_bir_lowering=False, debug=debug)

    nxb = (T + XB - 1) // XB
    xT_d = nc.dram_tensor("xT", [nxb, XB * B], dt, kind="ExternalInput")
    yT_d = nc.dram_tensor("yT", [T, B], dt, kind="ExternalOutput")
    A_ALL_d = nc.dram_tensor("A_ALL", [KR, 208], dt, kind="ExternalInput")

    SIG = mybir.ActivationFunctionType.Sigmoid
    TANH = mybir.ActivationFunctionType.Tanh
    MUL = mybir.AluOpType.mult
    SUB = mybir.AluOpType.subtract

    with tile.TileContext(nc) as tc:
        with (
            tc.tile_pool(name="wts", bufs=1) as wpool,
            tc.tile_pool(name="state", bufs=1) as stpool,
            tc.tile_pool(name="xin", bufs=2) as xpool,
            tc.tile_pool(name="sg", bufs=2) as spool,
            tc.tile_pool(name="tmp", bufs=2) as tpool,
            tc.tile_pool(name="ps", bufs=2, space=bass.MemorySpace.PSUM) as ppool,
        ):
            A_ALL = wpool.tile([KR, 208], dt, tag="aall")
            nc.sync.dma_start(A_ALL[:], A_ALL_d[:])

            ones = wpool.tile([1, B], dt, tag="ones")
            zrow = wpool.tile([1, B], dt, tag="zrow")
            nc.vector.memset(ones[:], 1.0)
            nc.vector.memset(zrow[:], 0.0)

            # R rotation, depth 4: rows 0:51 h1, 51 h2, 52 const-1, 64 x_t
            Rp = [stpool.tile([KR, B], dt, tag=f"R{k}", name=f"R{k}")
                  for k in range(4)]
            cc = stpool.tile([52, B], dt, tag="cc")
            for k in range(4):
                nc.vector.memset(Rp[k][:], 0.0)
            nc.vector.memset(cc[:], 0.0)
            for k in range(4):
                nc.sync.dma_start(Rp[k][52:53, :], ones[:])

            # x stripes live on partition XROW so a same-lane row copy feeds R
            stripes = {}

            def load_stripe(b):
                xs = xpool.tile([KR, XB * B], dt, tag="X")
                nc.sync.dma_start(xs[XROW:XROW + 1, :], xT_d[b:b + 1, :])
                stripes[b] = xs

            def xcopy(u):  # stage x_u into R[u%4] row 53
                xs = stripes[u // XB]
                nc.gpsimd.tensor_copy(
                    Rp[u % 4][XROW:XROW + 1, :],
                    xs[XROW:XROW + 1, (u % XB) * B:(u % XB + 1) * B])

            load_stripe(0)
            xcopy(0)
            xcopy(1)

            n_steps = T + 1  # device steps 0..T; layer 2 lags by one
            for s in range(n_steps):
                Rin = Rp[s % 4]
                Rout = Rp[(s + 1) % 4]

                # y row: Rin[51] = h2(s-2), written at step s-1, read here,
                # overwritten at step s+3 -> 3 steps of DMA slack
                if s >= 2:
                    nc.sync.dma_start(yT_d[s - 2:s - 1, :], Rin[51:52, :])

                # one PSUM bank per gate so each activation releases as soon
                # as its own matmul lands (PSUM deps are bank-granular)
                Pg = [ppool.tile([52, B], dt, tag=f"P{g}", name=f"P{g}")
                      for g in range(4)]
                for g in range(4):
                    nc.tensor.matmul(Pg[g][:],
                                     A_ALL[:, g * 52:(g + 1) * 52],
                                     Rin[:], start=True, stop=True)

                # blocks are [G, I, F, O]
                t_G = tpool.tile([52, B], dt, tag="tG")
                s_I = tpool.tile([52, B], dt, tag="sI")
                s_F = tpool.tile([52, B], dt, tag="sF")
                s_O = tpool.tile([52, B], dt, tag="sO")
                nc.scalar.activation(t_G[:], Pg[0][:], TANH)
                nc.scalar.activation(s_I[:], Pg[1][:], SIG)
                nc.scalar.activation(s_F[:], Pg[2][:], SIG)
                nc.scalar.activation(s_O[:], Pg[3][:], SIG)

                m = tpool.tile([52, B], dt, tag="m")
                t2 = tpool.tile([52, B], dt, tag="t2")
                tau = tpool.tile([52, B], dt, tag="tau")
                nc.vector.tensor_mul(m[:], s_I[:], t_G[:])
                nc.vector.tensor_mul(t2[:], s_F[:], cc[:])
                nc.vector.tensor_add(cc[:], m[:], t2[:])
                if s == 0:
                    nc.sync.dma_start(cc[51:52, :], zrow[:])  # c2 lag fix
                nc.scalar.activation(tau[:], cc[:], TANH)
                nc.vector.tensor_mul(Rout[0:52, :], s_O[:], tau[:])
                if s == 0:
                    nc.sync.dma_start(Rout[51:52, :], zrow[:])  # h2 lag fix

                # prefetch next stripe + stage x two steps ahead
                u = s + 2
                if u < T:
                    if u % XB == 0:
                        load_stripe(u // XB)
                    xcopy(u)

            # final row: y[T-1] = h2(T-1), in R[(T+1)%4][51] after step T
            nc.sync.dma_start(yT_d[T - 1:T, :], Rp[(T + 1) % 4][51:52, :])

    nc.compile()
    return nc


# ---------------------------------------------------------------------------
# Host-side execution: persistent jitted executable, device-resident weights,
# threaded shard fetch. Cached at module level so repeat kernel() calls are
# warm.
# ---------------------------------------------------------------------------

_RUNNER = None


class _Runner:
    def __init__(self, T):
        import jax
        from jax.sharding import Mesh, PartitionSpec, NamedSharding
        from jax.experimental.shard_map import shard_map
        import concourse.bass2jax as bass2jax
        from concourse import mybir

        self.T = T
        self.jax = jax
        nc = build_program(T=T)
        self.nc = nc
        bass2jax.install_neuronx_cc_hook()

        partition_name = (nc.partition_id_tensor.name
                          if nc.partition_id_tensor else None)
        in_names, out_names, out_avals, zero_outs = [], [], [], []
        for alloc in nc.m.functions[0].allocations:
            if not isinstance(alloc, mybir.MemoryLocationSet):
                continue
            name = alloc.memorylocations[0].name
            if alloc.kind == "ExternalInput":
                if name != partition_name:
                    in_names.append(name)
            elif alloc.kind == "ExternalOutput":
                shape = tuple(alloc.tensor_shape)
                dtype = mybir.dt.np(alloc.dtype)
                out_avals.append(jax.core.ShapedArray(shape, dtype))
                out_names.append(name)
                zero_outs.append(np.zeros(shape, dtype))
        self.in_names = in_names
        self.out_names = out_names
        n_params = len(in_names)
        n_outs = len(out_avals)
        in_names_all = in_names + out_names + (
            [partition_name] if partition_name else [])

        def _body(*args):
            operands = list(args)
            if partition_name is not None:
                operands.append(bass2jax.partition_id_tensor())
            return tuple(bass2jax._bass_exec_p.bind(
                *operands, out_avals=tuple(out_avals),
                in_names=tuple(in_names_all), out_names=tuple(out_names),
                lowering_input_output_aliases=(),
                sim_require_finite=True, sim_require_nnan=True, nc=nc))

        devices = jax.devices()[:NCORES]
        mesh = Mesh(np.asarray(devices), ("core",))
        self.sharding = NamedSharding(mesh, PartitionSpec("core"))
        # No donation: the kernel writes every yT element, so the zero
        # "output seed" buffers can live on device once and be reused.
        self.sharded = jax.jit(
            shard_map(_body, mesh=mesh,
                      in_specs=(PartitionSpec("core"),) * (n_params + n_outs),
                      out_specs=(PartitionSpec("core"),) * n_outs,
                      check_rep=False),
            keep_unused=True)
        self.dev_zeros = [
            jax.device_put(
                np.zeros((NCORES * z.shape[0], *z.shape[1:]), z.dtype),
                self.sharding)
            for z in zero_outs]
        jax.block_until_ready(self.dev_zeros)
        self._dev_aall = None

    def upload(self, xT, pk):
        """xT: (T, N) float32. Returns device input list (sharded)."""
        jax = self.jax
        T = self.T
        per_core_x = [
            np.ascontiguousarray(xT[:, c * B:(c + 1) * B]).reshape(-1, XB * B)
            for c in range(NCORES)]
        ins = {"xT": np.concatenate(per_core_x, axis=0),
               "A_ALL": np.concatenate([pk["A_ALL"]] * NCORES, axis=0)}
        dev = [jax.device_put(ins[n], self.sharding) for n in self.in_names]
        jax.block_until_ready(dev)
        return dev

    def execute(self, dev_in):
        out = self.sharded(*dev_in, *self.dev_zeros)
        self.jax.block_until_ready(out)
        return out

    def fetch(self, out):
        from concurrent.futures import ThreadPoolExecutor
        o = out[self.out_names.index("yT")]
        with ThreadPoolExecutor(NCORES) as ex:
            datas = list(ex.map(lambda sh: np.asarray(sh.data),
                                o.addressable_shards))
        yT = np.concatenate([d.reshape(self.T, B) for d in datas], axis=1)
        return yT


def get_runner(T=T_FULL):
    global _RUNNER
    if _RUNNER is None or _RUNNER.T != T:
        _RUNNER = _Runner(T)
    return _RUNNER


def kernel(stimulus, W_ih1, W_hh1, b_ih1, b_hh1, W_ih2, W_hh2, b_ih2, b_hh2):
    N, T = stimulus.shape
    assert (N, T) == (N_FULL, T_FULL)
    pk = pack_weights(W_ih1, W_hh1, b_ih1, b_hh1, W_ih2, W_hh2, b_ih2, b_hh2)
    xT = np.ascontiguousarray(stimulus.T.astype(np.float32))  # (T, N)
    r = get_runner(T)
    dev_in = r.upload(xT, pk)
    out = r.execute(dev_in)
    yT = r.fetch(out)
    return np.ascontiguousarray(yT.T)  # (N, T)


# revision 9
# speedup vs baseline: 2.3981x; 2.3981x over previous
"""Trainium2 Bass kernel for the 2-layer LSTM (H=51 -> H=1) over T=2048 steps.

Data-parallel over batch: 8 cores x 128 batch, batch on the free dim,
state transposed (units on partitions). Per core/step all gate
pre-activations for BOTH layers land in one PSUM tile P (52, 512) as four
column blocks [I|G|F|O]; block col 51 is the (one-step-lagged) layer-2 cell.
tanh(z) = 2*sigmoid(2z) - 1 with the x2 folded into G weights, so sigmoid
covers all gates; the sigmoid is split [I,G]/[F,O] so the i*tanh(g) chain
starts before the F/O half finishes.

x_t enters through the state tile itself: row 53 of R holds x_t, loaded by
a GpSimd row-copy from an SBUF stripe that lives on partition 53 (DMA'd
from HBM 64 steps at a time), so each gate needs ONE matmul (K=54) instead
of a rank-1 update + matmul. R rotates over 4 tiles for DMA slack; y rows
leave via direct SBUF->HBM DMA each step.
"""

import numpy as np

H = 51
B = 128
NCORES = 8
N_FULL = 1024
T_FULL = 2048
XB = 64          # time steps per X stripe

_GATES = [(slice(102, 153), 2, 1.0),   # G (tanh applied directly)
          (slice(0, 51), 0, 1.0),      # I
          (slice(51, 102), 1, 1.0),    # F
          (slice(153, 204), 3, 1.0)]   # O


KR = 65      # K rows of R/A_ALL; x sits at partition 64 (engine ops need
XROW = 64    # quadrant-aligned partition starts, so 53 is not addressable)


def pack_weights(W_ih1, W_hh1, b_ih1, b_hh1, W_ih2, W_hh2, b_ih2, b_hh2):
    """lhsT pack. K rows: 0:51 h1, 51 h2, 52 const-1(bias), 64 x_t (53:64
    zero pad). M cols: four 52-wide gate blocks [I|G|F|O]; block col 51 =
    layer-2 cell."""
    blocks = []
    for l1_rows, l2_row, scale in _GATES:
        L = np.zeros((KR, 52), np.float32)
        L[0:51, 0:51] = W_hh1[l1_rows, :].T
        L[0:51, 51] = W_ih2[l2_row, :]
        L[51, 51] = W_hh2[l2_row, 0]
        L[52, 0:51] = b_ih1[l1_rows] + b_hh1[l1_rows]
        L[52, 51] = b_ih2[l2_row] + b_hh2[l2_row]
        L[XROW, 0:51] = W_ih1[l1_rows, 0]
        blocks.append(L * scale)
    return {"A_ALL": np.concatenate(blocks, axis=1)}  # (KR, 208)


def build_program(T=T_FULL, debug=False):
    import concourse.bass as bass
    import concourse.tile as tile
    from concourse import bacc, mybir

    dt = mybir.dt.float32
    nc = bacc.Bacc("TRN2", target_bir_lowering=False, debug=debug)

    nxb = (T + XB - 1) // XB
    xT_d = nc.dram_tensor("xT", [nxb, XB * B], dt, kind="ExternalInput")
    yT_d = nc.dram_tensor("yT", [T, B], dt, kind="ExternalOutput")
    A_ALL_d = nc.dram_tensor("A_ALL", [KR, 208], dt, kind="ExternalInput")

    SIG = mybir.ActivationFunctionType.Sigmoid
    TANH = mybir.ActivationFunctionType.Tanh
    MUL = mybir.AluOpType.mult
    SUB = mybir.AluOpType.subtract

    with tile.TileContext(nc) as tc:
        with (
            tc.tile_pool(name="wts", bufs=1) as wpool,
            tc.tile_pool(name="state", bufs=1) as stpool,
            tc.tile_pool(name="xin", bufs=2) as xpool,
            tc.tile_pool(name="sg", bufs=2) as spool,
            tc.tile_pool(name="tmp", bufs=2) as tpool,
            tc.tile_pool(name="ps", bufs=2, space=bass.MemorySpace.PSUM) as ppool,
        ):
            A_ALL = wpool.tile([KR, 208], dt, tag="aall")
            nc.sync.dma_start(A_ALL[:], A_ALL_d[:])

            ones = wpool.tile([1, B], dt, tag="ones")
            zrow = wpool.tile([1, B], dt, tag="zrow")
            nc.vector.memset(ones[:], 1.0)
            nc.vector.memset(zrow[:], 0.0)

            # R rotation, depth 4: rows 0:51 h1, 51 h2, 52 const-1, 64 x_t
            Rp = [stpool.tile([KR, B], dt, tag=f"R{k}", name=f"R{k}")
                  for k in range(4)]
            cc = stpool.tile([52, B], dt, tag="cc")
            for k in range(4):
                nc.vector.memset(Rp[k][:], 0.0)
            nc.vector.memset(cc[:], 0.0)
            for k in range(4):
                nc.sync.dma_start(Rp[k][52:53, :], ones[:])

            # x stripes live on partition XROW so a same-lane row copy feeds R
            stripes = {}

            def load_stripe(b):
                # issue on the gpsimd SWDGE ring: keeps the 32KB single-
                # partition stripe transfer out of the SP HWDGE FIFO that
                # the per-step y-row DMAs (and their WAR releases) ride on
                xs = xpool.tile([KR, XB * B], dt, tag="X")
                nc.gpsimd.dma_start(xs[XROW:XROW + 1, :], xT_d[b:b + 1, :])
                stripes[b] = xs

            def xcopy(u):  # stage x_u into R[u%4] row 53
                xs = stripes[u // XB]
                nc.gpsimd.tensor_copy(
                    Rp[u % 4][XROW:XROW + 1, :],
                    xs[XROW:XROW + 1, (u % XB) * B:(u % XB + 1) * B])

            load_stripe(0)
            xcopy(0)
            xcopy(1)

            n_steps = T + 1  # device steps 0..T; layer 2 lags by one
            for s in range(n_steps):
                Rin = Rp[s % 4]
                Rout = Rp[(s + 1) % 4]

                # y row: Rin[51] = h2(s-2), written at step s-1, read here,
                # overwritten at step s+3 -> 3 steps of DMA slack
                if s >= 2:
                    nc.sync.dma_start(yT_d[s - 2:s - 1, :], Rin[51:52, :])

                # one PSUM bank per gate so each activation releases as soon
                # as its own matmul lands (PSUM deps are bank-granular)
                Pg = [ppool.tile([52, B], dt, tag=f"P{g}", name=f"P{g}")
                      for g in range(4)]
                for g in range(4):
                    nc.tensor.matmul(Pg[g][:],
                                     A_ALL[:, g * 52:(g + 1) * 52],
                                     Rin[:], start=True, stop=True)

                # blocks are [G, I, F, O]
                t_G = tpool.tile([52, B], dt, tag="tG")
                s_I = tpool.tile([52, B], dt, tag="sI")
                s_F = tpool.tile([52, B], dt, tag="sF")
                s_O = tpool.tile([52, B], dt, tag="sO")
                nc.scalar.activation(t_G[:], Pg[0][:], TANH)
                nc.scalar.activation(s_I[:], Pg[1][:], SIG)
                nc.scalar.activation(s_F[:], Pg[2][:], SIG)
                nc.scalar.activation(s_O[:], Pg[3][:], SIG)

                m = tpool.tile([52, B], dt, tag="m")
                t2 = tpool.tile([52, B], dt, tag="t2")
                tau = tpool.tile([52, B], dt, tag="tau")
                nc.vector.tensor_mul(m[:], s_I[:], t_G[:])
                nc.vector.tensor_mul(t2[:], s_F[:], cc[:])
                nc.vector.tensor_add(cc[:], m[:], t2[:])
                if s == 0:
                    nc.sync.dma_start(cc[51:52, :], zrow[:])  # c2 lag fix
                nc.scalar.activation(tau[:], cc[:], TANH)
                nc.vector.tensor_mul(Rout[0:52, :], s_O[:], tau[:])
                if s == 0:
                    nc.sync.dma_start(Rout[51:52, :], zrow[:])  # h2 lag fix

                # prefetch next stripe + stage x two steps ahead
                u = s + 2
                if u < T:
                    if u % XB == 0:
                        load_stripe(u // XB)
                    xcopy(u)

            # final row: y[T-1] = h2(T-1), in R[(T+1)%4][51] after step T
            nc.sync.dma_start(yT_d[T - 1:T, :], Rp[(T + 1) % 4][51:52, :])

    nc.compile()
    return nc


# ---------------------------------------------------------------------------
# Host-side execution: persistent jitted executable, device-resident weights,
# threaded shard fetch. Cached at module level so repeat kernel() calls are
# warm.
# ---------------------------------------------------------------------------

_RUNNER = None


class _Runner:
    def __init__(self, T):
        import jax
        from jax.sharding import Mesh, PartitionSpec, NamedSharding
        from jax.experimental.shard_map import shard_map
        import concourse.bass2jax as bass2jax
        from concourse import mybir

        self.T = T
        self.jax = jax
        nc = build_program(T=T)
        self.nc = nc
        bass2jax.install_neuronx_cc_hook()

        partition_name = (nc.partition_id_tensor.name
                          if nc.partition_id_tensor else None)
        in_names, out_names, out_avals, zero_outs = [], [], [], []
        for alloc in nc.m.functions[0].allocations:
            if not isinstance(alloc, mybir.MemoryLocationSet):
                continue
            name = alloc.memorylocations[0].name
            if alloc.kind == "ExternalInput":
                if name != partition_name:
                    in_names.append(name)
            elif alloc.kind == "ExternalOutput":
                shape = tuple(alloc.tensor_shape)
                dtype = mybir.dt.np(alloc.dtype)
                out_avals.append(jax.core.ShapedArray(shape, dtype))
                out_names.append(name)
                zero_outs.append(np.zeros(shape, dtype))
        self.in_names = in_names
        self.out_names = out_names
        n_params = len(in_names)
        n_outs = len(out_avals)
        in_names_all = in_names + out_names + (
            [partition_name] if partition_name else [])

        def _body(*args):
            operands = list(args)
            if partition_name is not None:
                operands.append(bass2jax.partition_id_tensor())
            return tuple(bass2jax._bass_exec_p.bind(
                *operands, out_avals=tuple(out_avals),
                in_names=tuple(in_names_all), out_names=tuple(out_names),
                lowering_input_output_aliases=(),
                sim_require_finite=True, sim_require_nnan=True, nc=nc))

        devices = jax.devices()[:NCORES]
        mesh = Mesh(np.asarray(devices), ("core",))
        self.sharding = NamedSharding(mesh, PartitionSpec("core"))
        # No donation: the kernel writes every yT element, so the zero
        # "output seed" buffers can live on device once and be reused.
        self.sharded = jax.jit(
            shard_map(_body, mesh=mesh,
                      in_specs=(PartitionSpec("core"),) * (n_params + n_outs),
                      out_specs=(PartitionSpec("core"),) * n_outs,
                      check_rep=False),
            keep_unused=True)
        self.dev_zeros = [
            jax.device_put(
                np.zeros((NCORES * z.shape[0], *z.shape[1:]), z.dtype),
                self.sharding)
            for z in zero_outs]
        jax.block_until_ready(self.dev_zeros)
        self._dev_aall = None

    def upload(self, xT, pk):
        """xT: (T, N) float32. Returns device input list (sharded)."""
        jax = self.jax
        T = self.T
        per_core_x = [
            np.ascontiguousarray(xT[:, c * B:(c + 1) * B]).reshape(-1, XB * B)
            for c in range(NCORES)]
        ins = {"xT": np.concatenate(per_core_x, axis=0),
               "A_ALL": np.concatenate([pk["A_ALL"]] * NCORES, axis=0)}
        dev = [jax.device_put(ins[n], self.sharding) for n in self.in_names]
        jax.block_until_ready(dev)
        return dev

    def execute(self, dev_in):
        out = self.sharded(*dev_in, *self.dev_zeros)
        self.jax.block_until_ready(out)
        return out

    def fetch(self, out):
        from concurrent.futures import ThreadPoolExecutor
        o = out[self.out_names.index("yT")]
        with ThreadPoolExecutor(NCORES) as ex:
            datas = list(ex.map(lambda sh: np.asarray(sh.data),
                                o.addressable_shards))
        yT = np.concatenate([d.reshape(self.T, B) for d in datas], axis=1)
        return yT


def get_runner(T=T_FULL):
    global _RUNNER
    if _RUNNER is None or _RUNNER.T != T:
        _RUNNER = _Runner(T)
    return _RUNNER


def kernel(stimulus, W_ih1, W_hh1, b_ih1, b_hh1, W_ih2, W_hh2, b_ih2, b_hh2):
    N, T = stimulus.shape
    assert (N, T) == (N_FULL, T_FULL)
    pk = pack_weights(W_ih1, W_hh1, b_ih1, b_hh1, W_ih2, W_hh2, b_ih2, b_hh2)
    xT = np.ascontiguousarray(stimulus.T.astype(np.float32))  # (T, N)
    r = get_runner(T)
    dev_in = r.upload(xT, pk)
    out = r.execute(dev_in)
    yT = r.fetch(out)
    return np.ascontiguousarray(yT.T)  # (N, T)


# revision 11
# speedup vs baseline: 2.4602x; 1.0259x over previous
"""Trainium2 Bass kernel for the 2-layer LSTM (H=51 -> H=1) over T=2048 steps.

Data-parallel over batch: 8 cores x 128 batch, batch on the free dim,
state transposed (units on partitions). Per core/step the four gate
pre-activations [G|I|F|O] for BOTH layers land in four per-gate PSUM banks
(52, 128) — PSUM deps are bank-granular, so each activation fires as soon
as its own matmul lands; the extra "unit" row 51 of every block is the
(one-step-lagged) layer-2 cell. ACT applies tanh(G)/sig(I)/sig(F)/sig(O)
then tanh(c); DVE does the four cell products/sums.

x_t enters through the state tile itself: row 64 of R holds x_t (engine
partition starts must be quadrant-aligned, so 64, with lhsT rows 53:64
zero), staged by a GpSimd same-lane row copy from an SBUF stripe living on
partition 64 (DMA'd from HBM 64 steps at a time on the gpsimd SWDGE ring,
off the y-DMA FIFO). Each gate thus needs ONE matmul (K=65) instead of a
rank-1 update + matmul. R rotates over 4 tiles so the per-step y-row
SBUF->HBM DMA gets 3 steps of WAR slack.
"""

import numpy as np

H = 51
B = 128
NCORES = 8
N_FULL = 1024
T_FULL = 2048
XB = 64          # time steps per X stripe

_GATES = [(slice(102, 153), 2, 1.0),   # G (tanh applied directly)
          (slice(0, 51), 0, 1.0),      # I
          (slice(51, 102), 1, 1.0),    # F
          (slice(153, 204), 3, 1.0)]   # O


KR = 65      # K rows of R/A_ALL; x sits at partition 64 (engine ops need
XROW = 64    # quadrant-aligned partition starts, so 53 is not addressable)


def pack_weights(W_ih1, W_hh1, b_ih1, b_hh1, W_ih2, W_hh2, b_ih2, b_hh2):
    """lhsT pack. K rows: 0:51 h1, 51 h2, 52 const-1(bias), 64 x_t (53:64
    zero pad). M cols: four 52-wide gate blocks [I|G|F|O]; block col 51 =
    layer-2 cell."""
    blocks = []
    for l1_rows, l2_row, scale in _GATES:
        L = np.zeros((KR, 52), np.float32)
        L[0:51, 0:51] = W_hh1[l1_rows, :].T
        L[0:51, 51] = W_ih2[l2_row, :]
        L[51, 51] = W_hh2[l2_row, 0]
        L[52, 0:51] = b_ih1[l1_rows] + b_hh1[l1_rows]
        L[52, 51] = b_ih2[l2_row] + b_hh2[l2_row]
        L[XROW, 0:51] = W_ih1[l1_rows, 0]
        blocks.append(L * scale)
    return {"A_ALL": np.concatenate(blocks, axis=1)}  # (KR, 208)


def build_program(T=T_FULL, debug=False):
    import concourse.bass as bass
    import concourse.tile as tile
    from concourse import bacc, mybir

    dt = mybir.dt.float32
    nc = bacc.Bacc("TRN2", target_bir_lowering=False, debug=debug)

    nxb = (T + XB - 1) // XB
    xT_d = nc.dram_tensor("xT", [nxb, XB * B], dt, kind="ExternalInput")
    yT_d = nc.dram_tensor("yT", [T, B], dt, kind="ExternalOutput")
    A_ALL_d = nc.dram_tensor("A_ALL", [KR, 208], dt, kind="ExternalInput")

    SIG = mybir.ActivationFunctionType.Sigmoid
    TANH = mybir.ActivationFunctionType.Tanh

    with tile.TileContext(nc) as tc:
        with (
            tc.tile_pool(name="wts", bufs=1) as wpool,
            tc.tile_pool(name="state", bufs=1) as stpool,
            tc.tile_pool(name="xin", bufs=2) as xpool,
            tc.tile_pool(name="tmp", bufs=2) as tpool,
            tc.tile_pool(name="ps", bufs=2, space=bass.MemorySpace.PSUM) as ppool,
        ):
            A_ALL = wpool.tile([KR, 208], dt, tag="aall")
            nc.sync.dma_start(A_ALL[:], A_ALL_d[:])

            ones = wpool.tile([1, B], dt, tag="ones")
            zrow = wpool.tile([1, B], dt, tag="zrow")
            nc.vector.memset(ones[:], 1.0)
            nc.vector.memset(zrow[:], 0.0)

            # R rotation, depth 4: rows 0:51 h1, 51 h2, 52 const-1, 64 x_t
            Rp = [stpool.tile([KR, B], dt, tag=f"R{k}", name=f"R{k}")
                  for k in range(4)]
            cc = stpool.tile([52, B], dt, tag="cc")
            for k in range(4):
                nc.vector.memset(Rp[k][:], 0.0)
            nc.vector.memset(cc[:], 0.0)
            for k in range(4):
                nc.sync.dma_start(Rp[k][52:53, :], ones[:])

            # x stripes live on partition XROW so a same-lane row copy feeds R
            stripes = {}

            def load_stripe(b):
                # issue on the gpsimd SWDGE ring: keeps the 32KB single-
                # partition stripe transfer out of the SP HWDGE FIFO that
                # the per-step y-row DMAs (and their WAR releases) ride on
                xs = xpool.tile([KR, XB * B], dt, tag="X")
                nc.gpsimd.dma_start(xs[XROW:XROW + 1, :], xT_d[b:b + 1, :])
                stripes[b] = xs

            def xcopy(u):  # stage x_u into R[u%4] row 53
                xs = stripes[u // XB]
                nc.gpsimd.tensor_copy(
                    Rp[u % 4][XROW:XROW + 1, :],
                    xs[XROW:XROW + 1, (u % XB) * B:(u % XB + 1) * B])

            load_stripe(0)
            xcopy(0)
            xcopy(1)

            n_steps = T + 1  # device steps 0..T; layer 2 lags by one
            for s in range(n_steps):
                Rin = Rp[s % 4]
                Rout = Rp[(s + 1) % 4]

                # y row: Rin[51] = h2(s-2), written at step s-1, read here,
                # overwritten at step s+3 -> 3 steps of DMA slack
                if s >= 2:
                    nc.sync.dma_start(yT_d[s - 2:s - 1, :], Rin[51:52, :])

                # one PSUM bank per gate so each activation releases as soon
                # as its own matmul lands (PSUM deps are bank-granular)
                Pg = [ppool.tile([52, B], dt, tag=f"P{g}", name=f"P{g}")
                      for g in range(4)]
                for g in range(4):
                    nc.tensor.matmul(Pg[g][:],
                                     A_ALL[:, g * 52:(g + 1) * 52],
                                     Rin[:], start=True, stop=True)

                # blocks are [G, I, F, O]
                t_G = tpool.tile([52, B], dt, tag="tG")
                s_I = tpool.tile([52, B], dt, tag="sI")
                s_F = tpool.tile([52, B], dt, tag="sF")
                s_O = tpool.tile([52, B], dt, tag="sO")
                nc.scalar.activation(t_G[:], Pg[0][:], TANH)
                nc.scalar.activation(s_I[:], Pg[1][:], SIG)
                nc.scalar.activation(s_F[:], Pg[2][:], SIG)
                nc.scalar.activation(s_O[:], Pg[3][:], SIG)

                m = tpool.tile([52, B], dt, tag="m")
                t2 = tpool.tile([52, B], dt, tag="t2")
                tau = tpool.tile([52, B], dt, tag="tau")
                nc.vector.tensor_mul(m[:], s_I[:], t_G[:])
                nc.vector.tensor_mul(t2[:], s_F[:], cc[:])
                nc.vector.tensor_add(cc[:], m[:], t2[:])
                if s == 0:
                    nc.sync.dma_start(cc[51:52, :], zrow[:])  # c2 lag fix
                nc.scalar.activation(tau[:], cc[:], TANH)
                nc.vector.tensor_mul(Rout[0:52, :], s_O[:], tau[:])
                if s == 0:
                    nc.sync.dma_start(Rout[51:52, :], zrow[:])  # h2 lag fix

                # prefetch next stripe + stage x two steps ahead
                u = s + 2
                if u < T:
                    if u % XB == 0:
                        load_stripe(u // XB)
                    xcopy(u)

            # final row: y[T-1] = h2(T-1), in R[(T+1)%4][51] after step T
            nc.sync.dma_start(yT_d[T - 1:T, :], Rp[(T + 1) % 4][51:52, :])

    nc.compile()
    return nc


# ---------------------------------------------------------------------------
# Host-side execution: persistent jitted executable, device-resident weights,
# threaded shard fetch. Cached at module level so repeat kernel() calls are
# warm.
# ---------------------------------------------------------------------------

_RUNNER = None


class _Runner:
    def __init__(self, T):
        import jax
        from jax.sharding import Mesh, PartitionSpec, NamedSharding
        from jax.experimental.shard_map import shard_map
        import concourse.bass2jax as bass2jax
        from concourse import mybir

        self.T = T
        self.jax = jax
        nc = build_program(T=T)
        self.nc = nc
        bass2jax.install_neuronx_cc_hook()

        partition_name = (nc.partition_id_tensor.name
                          if nc.partition_id_tensor else None)
        in_names, out_names, out_avals, zero_outs = [], [], [], []
        for alloc in nc.m.functions[0].allocations:
            if not isinstance(alloc, mybir.MemoryLocationSet):
                continue
            name = alloc.memorylocations[0].name
            if alloc.kind == "ExternalInput":
                if name != partition_name:
                    in_names.append(name)
            elif alloc.kind == "ExternalOutput":
                shape = tuple(alloc.tensor_shape)
                dtype = mybir.dt.np(alloc.dtype)
                out_avals.append(jax.core.ShapedArray(shape, dtype))
                out_names.append(name)
                zero_outs.append(np.zeros(shape, dtype))
        self.in_names = in_names
        self.out_names = out_names
        n_params = len(in_names)
        n_outs = len(out_avals)
        in_names_all = in_names + out_names + (
            [partition_name] if partition_name else [])

        def _body(*args):
            operands = list(args)
            if partition_name is not None:
                operands.append(bass2jax.partition_id_tensor())
            return tuple(bass2jax._bass_exec_p.bind(
                *operands, out_avals=tuple(out_avals),
                in_names=tuple(in_names_all), out_names=tuple(out_names),
                lowering_input_output_aliases=(),
                sim_require_finite=True, sim_require_nnan=True, nc=nc))

        devices = jax.devices()[:NCORES]
        mesh = Mesh(np.asarray(devices), ("core",))
        self.sharding = NamedSharding(mesh, PartitionSpec("core"))
        # No donation: the kernel writes every yT element, so the zero
        # "output seed" buffers can live on device once and be reused.
        self.sharded = jax.jit(
            shard_map(_body, mesh=mesh,
                      in_specs=(PartitionSpec("core"),) * (n_params + n_outs),
                      out_specs=(PartitionSpec("core"),) * n_outs,
                      check_rep=False),
            keep_unused=True)
        self.dev_zeros = [
            jax.device_put(
                np.zeros((NCORES * z.shape[0], *z.shape[1:]), z.dtype),
                self.sharding)
            for z in zero_outs]
        jax.block_until_ready(self.dev_zeros)
        self._dev_aall = None

    def upload(self, xT, pk):
        """xT: (T, N) float32. Returns device input list (sharded)."""
        jax = self.jax
        T = self.T
        per_core_x = [
            np.ascontiguousarray(xT[:, c * B:(c + 1) * B]).reshape(-1, XB * B)
            for c in range(NCORES)]
        ins = {"xT": np.concatenate(per_core_x, axis=0),
               "A_ALL": np.concatenate([pk["A_ALL"]] * NCORES, axis=0)}
        dev = [jax.device_put(ins[n], self.sharding) for n in self.in_names]
        jax.block_until_ready(dev)
        return dev

    def execute(self, dev_in):
        out = self.sharded(*dev_in, *self.dev_zeros)
        self.jax.block_until_ready(out)
        return out

    def fetch(self, out):
        from concurrent.futures import ThreadPoolExecutor
        o = out[self.out_names.index("yT")]
        with ThreadPoolExecutor(NCORES) as ex:
            datas = list(ex.map(lambda sh: np.asarray(sh.data),
                                o.addressable_shards))
        yT = np.concatenate([d.reshape(self.T, B) for d in datas], axis=1)
        return yT


def get_runner(T=T_FULL):
    global _RUNNER
    if _RUNNER is None or _RUNNER.T != T:
        _RUNNER = _Runner(T)
    return _RUNNER


def kernel(stimulus, W_ih1, W_hh1, b_ih1, b_hh1, W_ih2, W_hh2, b_ih2, b_hh2):
    N, T = stimulus.shape
    assert (N, T) == (N_FULL, T_FULL)
    pk = pack_weights(W_ih1, W_hh1, b_ih1, b_hh1, W_ih2, W_hh2, b_ih2, b_hh2)
    xT = np.ascontiguousarray(stimulus.T.astype(np.float32))  # (T, N)
    r = get_runner(T)
    dev_in = r.upload(xT, pk)
    out = r.execute(dev_in)
    yT = r.fetch(out)
    return np.ascontiguousarray(yT.T)  # (N, T)
